# revision 7
# baseline (speedup 1.0000x reference)
"""Trainium2 Bass kernel for nn_LinearLayer_45243185496808.

Computes out[b,o] = sum_i tanh(x[b,i]*t) * w[o,i], w = sum_p coef[o,i,p],
with B=131072, I=O=128, data-parallel over batch on 8 NeuronCores.

v2 pipeline (B_CORE=16384 rows/core), built from trace analysis of v1:
  - w is reduced over p and transposed on the host (it is a parameter
    reshape; 32 KiB f16 instead of 1 MiB f32 of per-core HBM traffic).
  - The HBM window is the whole game (~12.6 MiB/core @ ~400 GB/s measured
    = ~31.5 us, plus ~7.2 us fixed engine-start preamble + ~2.4 us
    epilogue). Compute is kept strictly off the critical path:
      * x streams in as f32 but lands f16 (gpsimd SWDGE casts in flight;
        halves SBUF-fabric bytes — measured faster than f32 loads).
      * first small chunk rides sync HWDGE (starts ~1.5 us before the
        first SWDGE chunk can), as f32 since HWDGE cannot cast.
      * per 128-row slice: PE transpose (f16 1 cyc/row) -> PSUM, ScalarE
        tanh -> SBUF f16, one f16 matmul (N=128, no weight duplication)
        -> PSUM f32, DVE strided copy -> f16 out tile.
      * no PE warmup: f16 work/slice fits the DMA feed rate even at the
        cold 1.2 GHz HAM clock.
  - All x/out SBUF tiles are resident simultaneously (no pool-reuse
    stalls); identity matrices come in as host inputs so gpsimd's
    instruction stream starts with the load dispatches.
  - stores ride sync HWDGE (measured better than gpsimd/scalar/split),
    last store is small so the tail is short.
Accuracy vs f64 reference ~1e-3 absmax-relative (f16 in/out + f16 mults).
"""

import os
import sys
import types

import numpy as np

import concourse.bass as bass
import concourse.mybir as mybir
import concourse.tile as tile
from concourse import bacc
from concourse.bass_utils import run_bass_kernel_spmd


def _ensure_ntff_hook():
    """Register the axon NTFF profile hook if the image lacks antenv.axon_hooks.

    Only needed for BASS_TRACE=1 profiling runs; harmless otherwise."""
    if "antenv.axon_hooks" in sys.modules:
        return
    try:
        from antenv.axon_hooks import get_axon_ntff_profile_hook  # noqa: F401

        return  # real module importable
    except ImportError:
        pass
    hook = None
    try:
        from trn_agent_boot.trn_boot import _ntff_profile_via_ctypes

        so_path = "/opt/axon/libaxon_pjrt.so"
        if os.path.exists(so_path):
            hook = _ntff_profile_via_ctypes(so_path)
    except Exception:
        hook = None
    mod = types.ModuleType("antenv.axon_hooks")
    mod.get_axon_ntff_profile_hook = lambda: hook
    mod.set_axon_ntff_profile_hook = lambda h: None
    sys.modules["antenv.axon_hooks"] = mod


N_CORES = 8
B_FULL = 131072
I_DIM = 128
O_DIM = 128
P_NUM = 16
P = 128                     # SBUF partitions
B_CORE = B_FULL // N_CORES  # 16384
G = 4                       # 128-row slices per PSUM group

# (row0, rows, is_first). First chunk is f32 on sync HWDGE; the rest are
# f32->f16 cast loads on gpsimd SWDGE, ~2 MiB reads (measured sweet spot).
CHUNK_PLAN = [(k * 2048, 2048, k == 0) for k in range(8)]
STORE_ROWS = 2048  # max rows per store piece (pieces never straddle chunks)

LAST_RESULT = None  # BassKernelResults of the most recent run (for test.py)


def build_bass(tanh_scale: float) -> bass.Bass:
    nc = bacc.Bacc("TRN2", target_bir_lowering=False)
    x = nc.dram_tensor("x", [B_CORE, I_DIM], mybir.dt.float32, kind="ExternalInput")
    wt = nc.dram_tensor("wt", [I_DIM, O_DIM], mybir.dt.float16, kind="ExternalInput")
    id16 = nc.dram_tensor("id16", [P, P], mybir.dt.float16, kind="ExternalInput")
    id32 = nc.dram_tensor("id32", [P, P], mybir.dt.float32, kind="ExternalInput")
    # Output leaves the device as f16 (halves store traffic; |out| << f16
    # range). Host upcasts back to f32.
    out = nc.dram_tensor("out", [B_CORE, O_DIM], mybir.dt.float16, kind="ExternalOutput")

    assert sum(r for _, r, _ in CHUNK_PLAN) == B_CORE
    assert all(a + r == b for (a, r, _), (b, _, _) in zip(CHUNK_PLAN, CHUNK_PLAN[1:]))

    def chunk_view(t, row0, rows):
        rpp = rows // P
        return t[row0 : row0 + rows, :].rearrange("(p r) d -> p (r d)", p=P)

    with tile.TileContext(nc) as tc:
        with (
            tc.tile_pool(name="consts", bufs=1) as consts,
            tc.tile_pool(name="xin", bufs=3) as xin_pool,
            tc.tile_pool(name="vals", bufs=4) as vals_pool,
            tc.tile_pool(name="outp", bufs=4) as out_pool,
            tc.tile_pool(name="pxT", bufs=3, space="PSUM") as pxT_pool,
            tc.tile_pool(name="pxT32", bufs=1, space="PSUM") as pxT32_pool,
            tc.tile_pool(name="pout", bufs=4, space="PSUM") as pout_pool,
        ):
            # --- load dispatches first: sync (chunk0 + consts), then SWDGE ---
            x_tiles = []
            for c, (row0, rows, first) in enumerate(CHUNK_PLAN):
                dt = mybir.dt.float32 if first else mybir.dt.float16
                tag = "x32" if first else "x_sb"
                x_sb = xin_pool.tile([P, (rows // P) * I_DIM], dt, tag=tag)
                x_tiles.append(x_sb)
                if first:
                    nc.sync.dma_start(out=x_sb[:], in_=chunk_view(x, row0, rows))

            wt_sb = consts.tile([P, O_DIM], mybir.dt.float16)
            nc.sync.dma_start(out=wt_sb[:], in_=wt[:, :])
            identity_h = consts.tile([P, P], mybir.dt.float16)
            nc.sync.dma_start(out=identity_h[:], in_=id16[:, :])
            identity_f = consts.tile([P, P], mybir.dt.float32)
            nc.sync.dma_start(out=identity_f[:], in_=id32[:, :])

            for c, (row0, rows, first) in enumerate(CHUNK_PLAN):
                if not first:
                    nc.gpsimd.dma_start(
                        out=x_tiles[c][:], in_=chunk_view(x, row0, rows)
                    )

            # --- main loop, software-pipelined on PE ---
            # PE's dispatch is strict FIFO: emitting T(g),M(g),T(g+1),... makes
            # PE idle-wait for ACT(g) before M(g). Emit T(g+1) before M(g) so
            # PE transposes the next group while ScalarE runs tanh on this one.
            groups = []  # (c, piece, g_in_piece, out_sb, x_sb, x_dt, ident)
            piece_meta = {}  # (c, pc) -> (out_view, piece_slices, n_groups_left)
            for c, (row0, rows, first) in enumerate(CHUNK_PLAN):
                n_slices = rows // P
                assert n_slices % G == 0
                n_pieces = -(-rows // STORE_ROWS)
                piece_slices = n_slices // n_pieces
                assert piece_slices * n_pieces == n_slices and piece_slices % G == 0
                out_view = chunk_view(out, row0, rows)
                for pc in range(n_pieces):
                    out_sb = out_pool.tile(
                        [P, piece_slices * O_DIM], mybir.dt.float16, tag="out_sb"
                    )
                    piece_meta[(c, pc)] = [out_view, piece_slices, piece_slices // G]
                    for g in range(piece_slices // G):
                        groups.append((c, pc, g, out_sb))

            n_groups = len(groups)
            stage = [None] * n_groups  # (xT_ps, v_T) per group

            def emit_front(gi):
                c, pc, g, out_sb = groups[gi]
                row0, rows, first = CHUNK_PLAN[c]
                x_sb = x_tiles[c]
                x_dt = mybir.dt.float32 if first else mybir.dt.float16
                ident = identity_f if first else identity_h
                piece_slices = piece_meta[(c, pc)][1]
                pool = pxT32_pool if first else pxT_pool
                tag = "xT32" if first else "xT_ps"
                xT_ps = pool.tile([P, G * P], x_dt, tag=tag)
                for j in range(G):
                    n = pc * piece_slices + g * G + j
                    nc.tensor.transpose(
                        xT_ps[:, j * P : (j + 1) * P],
                        x_sb[:, n * I_DIM : (n + 1) * I_DIM],
                        ident[:],
                    )
                v_T = vals_pool.tile([P, G * P], mybir.dt.float16)
                nc.scalar.activation(
                    v_T[:],
                    xT_ps[:],
                    mybir.ActivationFunctionType.Tanh,
                    scale=tanh_scale,
                )
                stage[gi] = v_T

            def emit_back(gi):
                c, pc, g, out_sb = groups[gi]
                v_T = stage[gi]
                o_ps = pout_pool.tile([P, G * O_DIM], mybir.dt.float32)
                for j in range(G):
                    nc.tensor.matmul(
                        o_ps[:, j * O_DIM : (j + 1) * O_DIM],
                        v_T[:, j * P : (j + 1) * P],
                        wt_sb[:],
                        start=True,
                        stop=True,
                    )
                nc.vector.tensor_copy(
                    out_sb[:, g * G * O_DIM : (g + 1) * G * O_DIM], o_ps[:]
                )
                meta = piece_meta[(c, pc)]
                meta[2] -= 1
                if meta[2] == 0:  # piece complete -> store it
                    out_view, piece_slices, _ = meta
                    nc.sync.dma_start(
                        out=out_view[
                            :,
                            pc * piece_slices * O_DIM : (pc + 1) * piece_slices * O_DIM,
                        ],
                        in_=out_sb[:],
                    )

            for gi in range(n_groups):
                emit_front(gi)
                if gi >= 1:
                    emit_back(gi - 1)
            emit_back(n_groups - 1)
    nc.finalize()
    return nc


def kernel(x, coef, tanh_range):
    global LAST_RESULT
    x = np.ascontiguousarray(np.asarray(x, dtype=np.float32))
    coef = np.asarray(coef, dtype=np.float32)
    t = float(np.asarray(tanh_range))
    assert x.shape == (B_FULL, I_DIM), x.shape
    assert coef.shape == (O_DIM, I_DIM, P_NUM), coef.shape

    # Parameter prep on host: w[o,i] = sum_p coef[o,i,p], laid out [i,o].
    wt = np.ascontiguousarray(coef.sum(axis=2).T.astype(np.float16))
    id16 = np.eye(P, dtype=np.float16)
    id32 = np.eye(P, dtype=np.float32)

    nc = build_bass(t)
    in_maps = [
        {
            "x": np.ascontiguousarray(x[k * B_CORE : (k + 1) * B_CORE]),
            "wt": wt,
            "id16": id16,
            "id32": id32,
        }
        for k in range(N_CORES)
    ]
    if os.environ.get("BASS_TRACE"):
        _ensure_ntff_hook()
    res = run_bass_kernel_spmd(nc, in_maps, core_ids=list(range(N_CORES)))
    LAST_RESULT = res
    return np.concatenate(
        [r["out"].astype(np.float32) for r in res.results], axis=0
    )
